# revision 8
# baseline (speedup 1.0000x reference)
"""CenterLoss (segment_reduce) Trainium2 kernel.

Math (faithful to the reference):
  preds = argmax_c logits[n, c, h, w]          (softmax is monotone -> skip it)
  s1[p] = sum_c x, s2[p] = sum_c x^2 per pixel p=(n,h,w)
  per (n, cls): cnt = #pixels with preds==cls, S1 = sum s1, S2 = sum s2
  K = max(cnt,1)*C; sq_dev = max(S2 - S1^2/K, 0)
  loss = sum_cls mean_n( cnt>0 ? sqrt(sq_dev) : 0 )

Device strategy (8 cores, data-parallel over 16 units = (n, H-slab of 128)):
  Each core takes 2 units of shape (C=19, 128, 1024) fp32.  SBUF layout puts
  H on partitions and (C, W) on the free dim, so per-pixel class reductions
  are free-dim ops at full 128-partition occupancy:
    m  = max over c   : pairwise TT tree, fp32 (exactness of the argmax mask)
    s1 = sum over c   : pairwise TT tree in bf16 (from an ACT bf16 cast)
    s2 = sum over c x^2: pairwise TT tree in bf16 (from ACT Square, bf16)
    per class c: STT (x_c ==) m   -> eq mask (bf16) + fused count accum
                 STT eq * s1      -> fused S1 accum
                 STT eq * s2      -> fused S2 accum
  Contiguous trees avoid the ~1.6 cyc/elem strided-read penalty of
  tensor_reduce with a strided innermost dim; the bf16 product path is
  eligible for the DVE 2x_1P mode.  Per-(partition, class) partial sums are
  DMA'd out; host sums the tiny partials and applies the final formula.
  `target` is unused by the reference and never shipped.
"""

import numpy as np

N, C, H, W = 4, 19, 512, 1024
NCORES = 8
SLABS = 4                 # H split into 4 slabs of 128 partitions
P = H // SLABS            # 128
UNITS = [(n, s) for n in range(N) for s in range(SLABS)]   # 16 units
UPC = len(UNITS) // NCORES                                  # 2 units per core
WCHUNK = 512
NCHUNKS = W // WCHUNK

_CACHE = {}


def _build_nc():
    from contextlib import ExitStack

    import concourse.tile as tile
    from concourse import bacc, mybir

    f32 = mybir.dt.float32
    bf16 = mybir.dt.bfloat16
    Alu = mybir.AluOpType
    Act = mybir.ActivationFunctionType

    nc = bacc.Bacc("TRN2", target_bir_lowering=False, debug=False)
    x_d = nc.dram_tensor("x", [UPC, C, P, W], f32, kind="ExternalInput").ap()
    out_d = nc.dram_tensor(
        "stats", [UPC, NCHUNKS, P, 3 * C], f32, kind="ExternalOutput"
    ).ap()

    with tile.TileContext(nc) as tc, ExitStack() as ctx:
        xpool = ctx.enter_context(tc.tile_pool(name="x", bufs=2))
        bfpool = ctx.enter_context(tc.tile_pool(name="bf", bufs=1))
        tpool = ctx.enter_context(tc.tile_pool(name="tree", bufs=1))
        eqpool = ctx.enter_context(tc.tile_pool(name="eq", bufs=3))
        jpool = ctx.enter_context(tc.tile_pool(name="junk", bufs=3))
        cpool = ctx.enter_context(tc.tile_pool(name="cols", bufs=2))

        def tree(src, op, dt, out_dt, tag):
            """Pairwise-reduce the C=19 rows of 3-dim tile `src` (P, 19, Wc)
            along the row dim via contiguous tensor_tensor ops, no copies:
            leftovers (src row 18, level-1 row 8) are folded in at the end.
            Intermediate levels use dtype dt; the final level writes a
            (P, WCHUNK) result of out_dt into the tree tile's row 9.
            Returns that AP."""
            assert C == 19
            t = tpool.tile([P, 10, WCHUNK], dt, tag=tag, name=f"tree_{tag}")
            if dt == out_dt:
                res = t[:, 9, :]
            else:
                res = tpool.tile(
                    [P, WCHUNK], out_dt, tag=tag + "o", name=f"tree_{tag}o"
                )[:]
            tt = nc.vector.tensor_tensor
            tt(out=t[:, 0:9, :], in0=src[:, 0:9, :], in1=src[:, 9:18, :], op=op)
            tt(out=t[:, 0:4, :], in0=t[:, 0:4, :], in1=t[:, 4:8, :], op=op)
            tt(out=t[:, 0:2, :], in0=t[:, 0:2, :], in1=t[:, 2:4, :], op=op)
            tt(out=t[:, 0, :], in0=t[:, 0, :], in1=t[:, 1, :], op=op)
            tt(out=t[:, 0, :], in0=t[:, 0, :], in1=t[:, 8, :], op=op)
            tt(out=res, in0=t[:, 0, :], in1=src[:, 18, :], op=op)
            return res

        for u in range(UPC):
            for ch in range(NCHUNKS):
                xt = xpool.tile([P, C, WCHUNK], f32, tag="x")
                src = x_d[u, :, :, ch * WCHUNK:(ch + 1) * WCHUNK]
                src = src.rearrange("c h w -> h c w")
                # Split the load so the max/sum trees (which read rows 0:18)
                # can start before row 18 lands.
                nc.sync.dma_start(xt[:, 0:9, :], src[:, 0:9, :])
                nc.sync.dma_start(xt[:, 9:18, :], src[:, 9:18, :])
                nc.sync.dma_start(xt[:, 18, :], src[:, 18, :])

                # bf16 casts on ScalarE (otherwise idle): x and x^2
                xb = bfpool.tile([P, C, WCHUNK], bf16, tag="xb")
                nc.scalar.activation(xb[:], xt[:], Act.Identity)
                sq = bfpool.tile([P, C, WCHUNK], bf16, tag="sq")
                nc.scalar.activation(sq[:], xt[:], Act.Square)

                m = tree(xt[:], Alu.max, f32, f32, "m")
                s1 = tree(xb[:], Alu.add, bf16, f32, "s1")
                s2 = tree(sq[:], Alu.add, bf16, f32, "s2")

                cols = cpool.tile([P, 3 * C], f32, tag="cols")
                for c in range(C):
                    eq = eqpool.tile([P, WCHUNK], f32, tag="eq")
                    nc.vector.scalar_tensor_tensor(
                        out=eq[:], in0=xt[:, c, :], scalar=1.0, in1=m,
                        op0=Alu.mult, op1=Alu.is_equal,
                        accum_out=cols[:, c:c + 1],
                    )
                    j1 = jpool.tile([P, WCHUNK], f32, tag="junk")
                    nc.vector.scalar_tensor_tensor(
                        out=j1[:], in0=eq[:], scalar=1.0, in1=s1,
                        op0=Alu.mult, op1=Alu.mult,
                        accum_out=cols[:, C + c:C + c + 1],
                    )
                    j2 = jpool.tile([P, WCHUNK], f32, tag="junk")
                    nc.vector.scalar_tensor_tensor(
                        out=j2[:], in0=eq[:], scalar=1.0, in1=s2,
                        op0=Alu.mult, op1=Alu.mult,
                        accum_out=cols[:, 2 * C + c:2 * C + c + 1],
                    )

                nc.sync.dma_start(out_d[u, ch], cols[:])

    nc.compile()
    return nc


def _get_nc():
    if "nc" not in _CACHE:
        _CACHE["nc"] = _build_nc()
    return _CACHE["nc"]


def _make_shards(logits):
    shards = []
    for k in range(NCORES):
        units = [UNITS[UPC * k + i] for i in range(UPC)]
        arr = np.stack(
            [logits[n, :, s * P:(s + 1) * P, :] for (n, s) in units]
        ).astype(np.float32, copy=False)
        shards.append(np.ascontiguousarray(arr))
    return shards


def _finish(results):
    per_n = np.zeros((N, 3, C), dtype=np.float64)
    for k in range(NCORES):
        arr = np.asarray(results[k]["stats"], dtype=np.float64)
        a = arr.reshape(UPC, NCHUNKS, P, 3, C).sum(axis=(1, 2))
        for ui in range(UPC):
            n, _s = UNITS[UPC * k + ui]
            per_n[n] += a[ui]
    cnt, S1, S2 = per_n[:, 0], per_n[:, 1], per_n[:, 2]
    K = np.maximum(cnt, 1.0) * C
    sq_dev = np.maximum(S2 - S1 * S1 / K, 0.0)
    norms = np.where(cnt > 0, np.sqrt(sq_dev), 0.0)
    loss = norms.mean(axis=0).sum()
    return np.array(loss, dtype=np.float32)


def kernel(**inputs):
    from concourse.bass_utils import run_bass_kernel_spmd

    logits = np.asarray(inputs["logits"])
    assert logits.shape == (N, C, H, W), logits.shape
    nc = _get_nc()
    shards = _make_shards(logits)
    in_maps = [{"x": shards[k]} for k in range(NCORES)]
    res = run_bass_kernel_spmd(nc, in_maps, list(range(NCORES)))
    return _finish(res.results)


# revision 10
# speedup vs baseline: 1.1822x; 1.1822x over previous
"""CenterLoss (segment_reduce) Trainium2 kernel.

Math (faithful to the reference):
  preds = argmax_c logits[n, c, h, w]          (softmax is monotone -> skip it)
  s1[p] = sum_c x, s2[p] = sum_c x^2 per pixel p=(n,h,w)
  per (n, cls): cnt = #pixels with preds==cls, S1 = sum s1, S2 = sum s2
  K = max(cnt,1)*C; sq_dev = max(S2 - S1^2/K, 0)
  loss = sum_cls mean_n( cnt>0 ? sqrt(sq_dev) : 0 )

Device strategy (8 cores, data-parallel over 16 units = (n, H-slab of 128)):
  Each core takes 2 units of shape (C=19, 128, 1024) fp32.  SBUF layout puts
  H on partitions and (C, W) on the free dim, so per-pixel class reductions
  are free-dim ops at full 128-partition occupancy:
    m  = max over c   : pairwise TT tree, fp32 (exactness of the argmax mask)
    s1 = sum over c   : pairwise TT tree in bf16 (from an ACT bf16 cast)
    s2 = sum over c x^2: pairwise TT tree in bf16 (from ACT Square, bf16)
    per class c: STT (x_c ==) m   -> eq mask (bf16) + fused count accum
                 STT eq * s1      -> fused S1 accum
                 STT eq * s2      -> fused S2 accum
  Contiguous trees avoid the ~1.6 cyc/elem strided-read penalty of
  tensor_reduce with a strided innermost dim; the bf16 product path is
  eligible for the DVE 2x_1P mode.  Per-(partition, class) partial sums are
  DMA'd out; host sums the tiny partials and applies the final formula.
  `target` is unused by the reference and never shipped.
"""

import numpy as np

N, C, H, W = 4, 19, 512, 1024
NCORES = 8
SLABS = 4                 # H split into 4 slabs of 128 partitions
P = H // SLABS            # 128
UNITS = [(n, s) for n in range(N) for s in range(SLABS)]   # 16 units
UPC = len(UNITS) // NCORES                                  # 2 units per core
WCHUNK = 512
NCHUNKS = W // WCHUNK

_CACHE = {}


def _build_nc():
    from contextlib import ExitStack

    import concourse.tile as tile
    from concourse import bacc, mybir

    f32 = mybir.dt.float32
    bf16 = mybir.dt.bfloat16
    Alu = mybir.AluOpType
    Act = mybir.ActivationFunctionType

    nc = bacc.Bacc("TRN2", target_bir_lowering=False, debug=False)
    x_d = nc.dram_tensor("x", [UPC, C, P, W], f32, kind="ExternalInput").ap()
    out_d = nc.dram_tensor(
        "stats", [UPC, NCHUNKS, P, 3 * C], f32, kind="ExternalOutput"
    ).ap()

    with tile.TileContext(nc) as tc, ExitStack() as ctx:
        xpool = ctx.enter_context(tc.tile_pool(name="x", bufs=2))
        bfpool = ctx.enter_context(tc.tile_pool(name="bf", bufs=1))
        tpool = ctx.enter_context(tc.tile_pool(name="tree", bufs=1))
        eqpool = ctx.enter_context(tc.tile_pool(name="eq", bufs=4))
        jpool = ctx.enter_context(tc.tile_pool(name="junk", bufs=4))
        cpool = ctx.enter_context(tc.tile_pool(name="cols", bufs=2))

        def tree(src, op, dt, out_dt, tag):
            """Pairwise-reduce the C=19 rows of 3-dim tile `src` (P, 19, Wc)
            along the row dim via contiguous tensor_tensor ops, no copies:
            leftovers (src row 18, level-1 row 8) are folded in at the end.
            Intermediate levels use dtype dt; the final level writes a
            (P, WCHUNK) result of out_dt into the tree tile's row 9.
            Returns that AP."""
            assert C == 19
            t = tpool.tile([P, 10, WCHUNK], dt, tag=tag, name=f"tree_{tag}")
            if dt == out_dt:
                res = t[:, 9, :]
            else:
                res = tpool.tile(
                    [P, WCHUNK], out_dt, tag=tag + "o", name=f"tree_{tag}o"
                )[:]
            tt = nc.vector.tensor_tensor
            tt(out=t[:, 0:9, :], in0=src[:, 0:9, :], in1=src[:, 9:18, :], op=op)
            tt(out=t[:, 0:4, :], in0=t[:, 0:4, :], in1=t[:, 4:8, :], op=op)
            tt(out=t[:, 0:2, :], in0=t[:, 0:2, :], in1=t[:, 2:4, :], op=op)
            tt(out=t[:, 0, :], in0=t[:, 0, :], in1=t[:, 1, :], op=op)
            tt(out=t[:, 0, :], in0=t[:, 0, :], in1=t[:, 8, :], op=op)
            tt(out=res, in0=t[:, 0, :], in1=src[:, 18, :], op=op)
            return res

        for u in range(UPC):
            for ch in range(NCHUNKS):
                xt = xpool.tile([P, C, WCHUNK], f32, tag="x")
                src = x_d[u, :, :, ch * WCHUNK:(ch + 1) * WCHUNK]
                nc.sync.dma_start(xt[:], src.rearrange("c h w -> h c w"))

                # bf16 casts on ScalarE (otherwise idle): x and x^2
                xb = bfpool.tile([P, C, WCHUNK], bf16, tag="xb")
                nc.scalar.activation(xb[:], xt[:], Act.Identity)
                sq = bfpool.tile([P, C, WCHUNK], bf16, tag="sq")
                nc.scalar.activation(sq[:], xt[:], Act.Square)

                m = tree(xt[:], Alu.max, f32, f32, "m")
                s1 = tree(xb[:], Alu.add, bf16, f32, "s1")
                s2 = tree(sq[:], Alu.add, bf16, f32, "s2")

                cols = cpool.tile([P, 3 * C], f32, tag="cols")
                for c in range(C):
                    eq = eqpool.tile([P, WCHUNK], f32, tag="eq")
                    nc.vector.scalar_tensor_tensor(
                        out=eq[:], in0=xt[:, c, :], scalar=1.0, in1=m,
                        op0=Alu.mult, op1=Alu.is_equal,
                        accum_out=cols[:, c:c + 1],
                    )
                    j1 = jpool.tile([P, WCHUNK], f32, tag="junk")
                    nc.vector.scalar_tensor_tensor(
                        out=j1[:], in0=eq[:], scalar=1.0, in1=s1,
                        op0=Alu.mult, op1=Alu.mult,
                        accum_out=cols[:, C + c:C + c + 1],
                    )
                    j2 = jpool.tile([P, WCHUNK], f32, tag="junk")
                    nc.vector.scalar_tensor_tensor(
                        out=j2[:], in0=eq[:], scalar=1.0, in1=s2,
                        op0=Alu.mult, op1=Alu.mult,
                        accum_out=cols[:, 2 * C + c:2 * C + c + 1],
                    )

                nc.sync.dma_start(out_d[u, ch], cols[:])

    nc.compile()
    return nc


def _get_nc():
    if "nc" not in _CACHE:
        _CACHE["nc"] = _build_nc()
    return _CACHE["nc"]


def _make_shards(logits):
    shards = []
    for k in range(NCORES):
        units = [UNITS[UPC * k + i] for i in range(UPC)]
        arr = np.stack(
            [logits[n, :, s * P:(s + 1) * P, :] for (n, s) in units]
        ).astype(np.float32, copy=False)
        shards.append(np.ascontiguousarray(arr))
    return shards


def _finish(results):
    per_n = np.zeros((N, 3, C), dtype=np.float64)
    for k in range(NCORES):
        arr = np.asarray(results[k]["stats"], dtype=np.float64)
        a = arr.reshape(UPC, NCHUNKS, P, 3, C).sum(axis=(1, 2))
        for ui in range(UPC):
            n, _s = UNITS[UPC * k + ui]
            per_n[n] += a[ui]
    cnt, S1, S2 = per_n[:, 0], per_n[:, 1], per_n[:, 2]
    K = np.maximum(cnt, 1.0) * C
    sq_dev = np.maximum(S2 - S1 * S1 / K, 0.0)
    norms = np.where(cnt > 0, np.sqrt(sq_dev), 0.0)
    loss = norms.mean(axis=0).sum()
    return np.array(loss, dtype=np.float32)


def kernel(**inputs):
    from concourse.bass_utils import run_bass_kernel_spmd

    logits = np.asarray(inputs["logits"])
    assert logits.shape == (N, C, H, W), logits.shape
    nc = _get_nc()
    shards = _make_shards(logits)
    in_maps = [{"x": shards[k]} for k in range(NCORES)]
    res = run_bass_kernel_spmd(nc, in_maps, list(range(NCORES)))
    return _finish(res.results)


# revision 12
# speedup vs baseline: 1.1983x; 1.0136x over previous
"""CenterLoss (segment_reduce) Trainium2 kernel.

Math (faithful to the reference):
  preds = argmax_c logits[n, c, h, w]          (softmax is monotone -> skip it)
  s1[p] = sum_c x, s2[p] = sum_c x^2 per pixel p=(n,h,w)
  per (n, cls): cnt = #pixels with preds==cls, S1 = sum s1, S2 = sum s2
  K = max(cnt,1)*C; sq_dev = max(S2 - S1^2/K, 0)
  loss = sum_cls mean_n( cnt>0 ? sqrt(sq_dev) : 0 )

Device strategy (8 cores, data-parallel over 16 units = (n, H-slab of 128)):
  Each core takes 2 units of shape (C=19, 128, 1024) fp32.  SBUF layout puts
  H on partitions and (C, W) on the free dim, so per-pixel class reductions
  are free-dim ops at full 128-partition occupancy:
    m  = max over c   : pairwise TT tree, fp32 (exactness of the argmax mask)
    s1 = sum over c   : pairwise TT tree in bf16 (from an ACT bf16 cast)
    s2 = sum over c x^2: pairwise TT tree in bf16 (from ACT Square, bf16)
    per class c: STT (x_c ==) m   -> eq mask + fused count accum
                 STT eq * s1      -> fused S1 accum
                 STT eq * s2      -> fused S2 accum
  Contiguous trees avoid the ~1.6 cyc/elem strided-read penalty of
  tensor_reduce with a strided innermost dim; bf16 tree levels ride the DVE
  2x_1P tensor_tensor mode.  The STT passes are fp32 (the fused
  scalar_tensor_tensor opcode has no fast mode, and bf16 outputs measured
  slower).  Per-(partition, class) partial sums are DMA'd out; host sums the
  tiny partials and applies the final formula.  `target` is unused by the
  reference and never shipped.

  Measured on trn2 via axon: ~255 us HW exec (DVE-bound at ~87% busy;
  HBM roofline for the 19.9 MB/core shard is ~55 us).
"""

import numpy as np


def _ensure_ntff_hook():
    """bass_utils' trace path imports antenv.axon_hooks, which this image
    lacks.  Install a shim backed by trn_agent_boot's ctypes hook so a
    BASS_TRACE=1 environment doesn't crash the run (and tracing works)."""
    import sys
    import types

    try:
        import antenv.axon_hooks  # noqa: F401
        return
    except ImportError:
        pass
    try:
        from trn_agent_boot.trn_boot import _ntff_profile_via_ctypes

        hook = _ntff_profile_via_ctypes("/opt/axon/libaxon_pjrt.so")
    except Exception:
        hook = None
    mod = types.ModuleType("antenv.axon_hooks")
    mod.get_axon_ntff_profile_hook = lambda: hook
    mod.set_axon_ntff_profile_hook = lambda h: None
    sys.modules["antenv.axon_hooks"] = mod

N, C, H, W = 4, 19, 512, 1024
NCORES = 8
SLABS = 4                 # H split into 4 slabs of 128 partitions
P = H // SLABS            # 128
UNITS = [(n, s) for n in range(N) for s in range(SLABS)]   # 16 units
UPC = len(UNITS) // NCORES                                  # 2 units per core
WCHUNK = 512
NCHUNKS = W // WCHUNK

_CACHE = {}


def _build_nc():
    from contextlib import ExitStack

    import concourse.tile as tile
    from concourse import bacc, mybir

    f32 = mybir.dt.float32
    bf16 = mybir.dt.bfloat16
    Alu = mybir.AluOpType
    Act = mybir.ActivationFunctionType

    nc = bacc.Bacc("TRN2", target_bir_lowering=False, debug=False)
    x_d = nc.dram_tensor("x", [UPC, C, P, W], f32, kind="ExternalInput").ap()
    out_d = nc.dram_tensor(
        "stats", [UPC, NCHUNKS, P, 3 * C], f32, kind="ExternalOutput"
    ).ap()

    with tile.TileContext(nc) as tc, ExitStack() as ctx:
        xpool = ctx.enter_context(tc.tile_pool(name="x", bufs=2))
        bfpool = ctx.enter_context(tc.tile_pool(name="bf", bufs=1))
        tpool = ctx.enter_context(tc.tile_pool(name="tree", bufs=1))
        eqpool = ctx.enter_context(tc.tile_pool(name="eq", bufs=4))
        jpool = ctx.enter_context(tc.tile_pool(name="junk", bufs=4))
        cpool = ctx.enter_context(tc.tile_pool(name="cols", bufs=2))

        def tree(src, op, dt, out_dt, tag):
            """Pairwise-reduce the C=19 rows of 3-dim tile `src` (P, 19, Wc)
            along the row dim via contiguous tensor_tensor ops, no copies:
            leftovers (src row 18, level-1 row 8) are folded in at the end.
            Intermediate levels use dtype dt; the final level writes a
            (P, WCHUNK) result of out_dt into the tree tile's row 9.
            Returns that AP."""
            assert C == 19
            t = tpool.tile([P, 10, WCHUNK], dt, tag=tag, name=f"tree_{tag}")
            if dt == out_dt:
                res = t[:, 9, :]
            else:
                res = tpool.tile(
                    [P, WCHUNK], out_dt, tag=tag + "o", name=f"tree_{tag}o"
                )[:]
            tt = nc.vector.tensor_tensor
            tt(out=t[:, 0:9, :], in0=src[:, 0:9, :], in1=src[:, 9:18, :], op=op)
            tt(out=t[:, 0:4, :], in0=t[:, 0:4, :], in1=t[:, 4:8, :], op=op)
            tt(out=t[:, 0:2, :], in0=t[:, 0:2, :], in1=t[:, 2:4, :], op=op)
            tt(out=t[:, 0, :], in0=t[:, 0, :], in1=t[:, 1, :], op=op)
            tt(out=t[:, 0, :], in0=t[:, 0, :], in1=t[:, 8, :], op=op)
            tt(out=res, in0=t[:, 0, :], in1=src[:, 18, :], op=op)
            return res

        for u in range(UPC):
            for ch in range(NCHUNKS):
                xt = xpool.tile([P, C, WCHUNK], f32, tag="x")
                src = x_d[u, :, :, ch * WCHUNK:(ch + 1) * WCHUNK]
                nc.sync.dma_start(xt[:], src.rearrange("c h w -> h c w"))

                # bf16 casts on ScalarE (otherwise idle): x and x^2
                xb = bfpool.tile([P, C, WCHUNK], bf16, tag="xb")
                nc.scalar.activation(xb[:], xt[:], Act.Identity)
                sq = bfpool.tile([P, C, WCHUNK], bf16, tag="sq")
                nc.scalar.activation(sq[:], xt[:], Act.Square)

                m = tree(xt[:], Alu.max, f32, f32, "m")
                s1 = tree(xb[:], Alu.add, bf16, f32, "s1")
                s2 = tree(sq[:], Alu.add, bf16, f32, "s2")

                cols = cpool.tile([P, 3 * C], f32, tag="cols")
                for c in range(C):
                    eq = eqpool.tile([P, WCHUNK], f32, tag="eq")
                    nc.vector.scalar_tensor_tensor(
                        out=eq[:], in0=xt[:, c, :], scalar=1.0, in1=m,
                        op0=Alu.mult, op1=Alu.is_equal,
                        accum_out=cols[:, c:c + 1],
                    )
                    j1 = jpool.tile([P, WCHUNK], f32, tag="junk")
                    nc.vector.scalar_tensor_tensor(
                        out=j1[:], in0=eq[:], scalar=1.0, in1=s1,
                        op0=Alu.mult, op1=Alu.mult,
                        accum_out=cols[:, C + c:C + c + 1],
                    )
                    j2 = jpool.tile([P, WCHUNK], f32, tag="junk")
                    nc.vector.scalar_tensor_tensor(
                        out=j2[:], in0=eq[:], scalar=1.0, in1=s2,
                        op0=Alu.mult, op1=Alu.mult,
                        accum_out=cols[:, 2 * C + c:2 * C + c + 1],
                    )

                nc.sync.dma_start(out_d[u, ch], cols[:])

    nc.compile()
    return nc


def _get_nc():
    if "nc" not in _CACHE:
        _CACHE["nc"] = _build_nc()
    return _CACHE["nc"]


def _make_shards(logits):
    shards = []
    for k in range(NCORES):
        units = [UNITS[UPC * k + i] for i in range(UPC)]
        arr = np.stack(
            [logits[n, :, s * P:(s + 1) * P, :] for (n, s) in units]
        ).astype(np.float32, copy=False)
        shards.append(np.ascontiguousarray(arr))
    return shards


def _finish(results):
    per_n = np.zeros((N, 3, C), dtype=np.float64)
    for k in range(NCORES):
        arr = np.asarray(results[k]["stats"], dtype=np.float64)
        a = arr.reshape(UPC, NCHUNKS, P, 3, C).sum(axis=(1, 2))
        for ui in range(UPC):
            n, _s = UNITS[UPC * k + ui]
            per_n[n] += a[ui]
    cnt, S1, S2 = per_n[:, 0], per_n[:, 1], per_n[:, 2]
    K = np.maximum(cnt, 1.0) * C
    sq_dev = np.maximum(S2 - S1 * S1 / K, 0.0)
    norms = np.where(cnt > 0, np.sqrt(sq_dev), 0.0)
    loss = norms.mean(axis=0).sum()
    return np.array(loss, dtype=np.float32)


def kernel(**inputs):
    _ensure_ntff_hook()
    from concourse.bass_utils import run_bass_kernel_spmd

    logits = np.asarray(inputs["logits"])
    assert logits.shape == (N, C, H, W), logits.shape
    nc = _get_nc()
    shards = _make_shards(logits)
    in_maps = [{"x": shards[k]} for k in range(NCORES)]
    res = run_bass_kernel_spmd(nc, in_maps, list(range(NCORES)))
    return _finish(res.results)
